# revision 27
# baseline (speedup 1.0000x reference)
"""Trainium2 Bass kernel for nn_AutoregressiveMixerBlock.

Reference computation (per batch b):
  y  = LN_H(x)                                    # layer norm over H
  t  = revcumsum_N(y)                             # t[j] = sum_{i>=j} y[i]
  h  = gelu(t^T @ tok_w1 + tok_b1)                # [H, TM]
  y2 = (h @ tok_w2 + tok_b2)^T                    # [N, H]
  y3 = LN_H(y2)
  out = gelu(y3 @ ch_w1 + ch_b1) @ ch_w2 + ch_b2  # [N, H]

Algebraic folds (exact in real arithmetic, applied on host):
  * revcumsum+matmul:  sum_j t[j,h] w1[j,m] = sum_i y[i,h] W1c[i,m]
    with W1c = cumsum(tok_w1, axis=0) -> no on-device cumsum at all.
  * LN1 gain/bias move past the token matmul.
  * tok_b2 and the LN2 mean both vanish by centering h^T by its
    per-row (over H) mean before the second token matmul.
  * LN2 gain/bias fold into ch_w1 / ch_b1.

Device strategy (per core, 2 batches):
  * all matmul operands bf16 (PE 1 cyc/row), f32 PSUM accumulate.
  * LN1 stats WITHOUT vector-engine reductions: a DMA-transposed copy
    of x ([H, N] layout) is squared on the scalar engine, and both
    sum_h and sum_h^2 per token come from tiny ones-column matmuls
    (contraction over partitions = H) into compact psum columns.
  * rsqrt everywhere via the fast-inverse-sqrt int hack (int ops on
    DVE, Newton multiplies on GPSIMD) -> the scalar engine only ever
    runs Gelu/Square/Copy/Identity = ONE activation table set.
  * LN2 variance reduced the same way (ones-column matmuls on the
    squared token-mix output); the per-token rstd is broadcast to a
    [128, N] bf16 tile via a DRAM roundtrip (store / transposed load /
    store / stride-0 broadcast load).
  * channel MLP output accumulates into bank 0 of the gelu-input psum
    tile (already consumed) and the two batches' channel-MLP sweeps
    run back-to-back with two alternating 4-bank psum tiles.
"""

import numpy as np

B, N, H = 16, 8192, 128
TM, CM = 256, 512
EPS = 1e-5
NCORES = 8
BL = B // NCORES          # batches per core
P = 128                   # partitions
NC_TOK = N // P           # 64 token chunks of 128
NJ = 16                   # j chunks per batch
JW = N // NJ              # 512 tokens per j chunk
TPJ = JW // P             # 4 token chunks per j chunk
KTM = TM // P             # 2 k-chunks for the second token matmul
NCI = CM // P             # 4 chunks of the channel hidden dim
MAGIC1 = 0x5F3759DF + 1   # fast-rsqrt magic (for the xor/asr variant)

_cached = {}


def _build(nontrivial_bias1, nontrivial_cb2):
    import contextlib

    import concourse.mybir as mybir
    import concourse.tile as tile
    from concourse import bacc
    from concourse.masks import make_identity

    F32 = mybir.dt.float32
    BF16 = mybir.dt.bfloat16
    I32 = mybir.dt.int32
    AF = mybir.ActivationFunctionType
    ALU = mybir.AluOpType

    nc = bacc.Bacc()

    # ---- DRAM tensors -------------------------------------------------
    x_d = nc.dram_tensor("x", [BL, N, H], BF16, kind="ExternalInput")
    w1c_d = nc.dram_tensor("w1c", [N, TM], BF16, kind="ExternalInput")
    w2_d = nc.dram_tensor("w2", [TM, N], BF16, kind="ExternalInput")
    g1_d = nc.dram_tensor("g1", [P, 1], F32, kind="ExternalInput")
    bias1_d = nc.dram_tensor("bias1", [P, TM], F32, kind="ExternalInput")
    cw1_d = nc.dram_tensor("cw1", [H, CM], BF16, kind="ExternalInput")
    cb1_d = nc.dram_tensor("cb1", [P, NCI], F32, kind="ExternalInput")
    cw2_d = nc.dram_tensor("cw2", [CM, H], BF16, kind="ExternalInput")
    cb2_d = nc.dram_tensor("cb2", [P, 1], F32, kind="ExternalInput")
    onesc_d = nc.dram_tensor("onesc", [P, 1], BF16, kind="ExternalInput")
    # rstd scratch: [t, c] then [c, t] layouts (DMA-transposed between)
    rs1_d = nc.dram_tensor("rs1", [BL, P, P], BF16, kind="ExternalOutput")
    rs2_d = nc.dram_tensor("rs2", [BL, NC_TOK, P], BF16,
                           kind="ExternalOutput")
    out_d = nc.dram_tensor("out", [BL, H, N], F32, kind="ExternalOutput")

    # DRAM views
    x_v = [x_d[b].rearrange("(c p) h -> p c h", p=P) for b in range(BL)]
    w1c_v = w1c_d[:].rearrange("(c p) m -> p c m", p=P)
    w2_v = w2_d[:].rearrange("(k p) (j n) -> p k j n", p=P, n=JW)
    cw2_v = cw2_d[:].rearrange("(ci p) h -> p ci h", p=P)
    out_v = [out_d[b] for b in range(BL)]

    with tile.TileContext(nc) as tc:
        with contextlib.ExitStack() as ctx:
            const = ctx.enter_context(tc.tile_pool(name="const", bufs=1))
            big = ctx.enter_context(tc.tile_pool(name="big", bufs=1))
            stat = ctx.enter_context(tc.tile_pool(name="stat", bufs=1))
            small = ctx.enter_context(tc.tile_pool(name="small", bufs=4))
            sqp = ctx.enter_context(tc.tile_pool(name="sqp", bufs=3))
            ynp = ctx.enter_context(tc.tile_pool(name="ynp", bufs=3))
            g2p = ctx.enter_context(tc.tile_pool(name="g2p", bufs=2))
            outp = ctx.enter_context(tc.tile_pool(name="outp", bufs=3))

            # big retained tiles; bcast[b] doubles as the transposed-x
            # scratch and y2r[b] as the squared-x scratch during LN1
            # (both are otherwise unused until phase 3)
            y2r = [big.tile([P, N], BF16, name=f"y2r{b}")
                   for b in range(BL)]
            bcast = [big.tile([P, N], BF16, name=f"bc{b}")
                     for b in range(BL)]
            xT_sb = bcast

            # ---- input DMAs, ordered by when they gate compute --------
            # (the sync engine issues these serially: transposed-x and
            # the tiny ones-column first — they gate the LN1 stat
            # matmuls — then the rest by first use)
            x_sb = [big.tile([P, NC_TOK, H], BF16, name=f"x{b}")
                    for b in range(BL)]
            nc.sync.dma_start(xT_sb[0], x_d[0], transpose=True)
            onesc_sb = const.tile([P, 1], BF16)
            nc.sync.dma_start(onesc_sb, onesc_d[:])
            g1_sb = const.tile([P, 1], F32)
            nc.sync.dma_start(g1_sb, g1_d[:])
            ident = const.tile([P, P], BF16)
            make_identity(nc, ident)
            nc.sync.dma_start(xT_sb[1], x_d[1], transpose=True)
            cw1_sb = const.tile([H, CM], BF16)
            nc.sync.dma_start(cw1_sb, cw1_d[:])
            cb1_sb = const.tile([P, NCI], F32)
            nc.sync.dma_start(cb1_sb, cb1_d[:])
            cw2_sb = const.tile([P, NCI, H], BF16)
            nc.sync.dma_start(cw2_sb, cw2_v)
            if nontrivial_bias1:
                bias1_sb = const.tile([P, TM], F32)
                nc.sync.dma_start(bias1_sb, bias1_d[:])
            if nontrivial_cb2:
                cb2_sb = const.tile([P, 1], F32)
                nc.sync.dma_start(cb2_sb, cb2_d[:])
            nc.sync.dma_start(x_sb[0], x_v[0])
            nc.sync.dma_start(x_sb[1], x_v[1])
            w1c_sb = big.tile([P, NC_TOK, TM], BF16, name="w1c")
            nc.sync.dma_start(w1c_sb, w1c_v)
            w2_sb = big.tile([P, KTM, NJ, JW], BF16, name="w2")
            nc.sync.dma_start(w2_sb, w2_v)
            g1_t = small.tile([P, 1], F32)
            nc.vector.tensor_copy(g1_t, g1_sb)

            # ---------------------------------------------------------
            def rsqrt_chain(dst, vsrc, tmp_a, tmp_b):
                """dst = 1/sqrt(vsrc) (vsrc f32 SBUF [P, n]).

                Fast-inverse-sqrt seed (int ops on the DVE's fp32-ALU
                int path, accurate to ~1e-6 here) + 2 Newton steps, all
                on DVE: the tiles are tiny so cross-engine handoff
                latency would dominate any offload. tmp_a/tmp_b f32 same
                shape; dst may be bf16.
                """
                iv = vsrc[:].bitcast(I32)
                nc.vector.tensor_scalar(
                    out=tmp_a[:].bitcast(I32), in0=iv, scalar1=-1, scalar2=1,
                    op0=ALU.bitwise_xor, op1=ALU.arith_shift_right)
                nc.vector.tensor_scalar(
                    out=tmp_b[:].bitcast(I32), in0=tmp_a[:].bitcast(I32),
                    scalar1=MAGIC1, scalar2=None, op0=ALU.add)
                for it in range(2):
                    nc.vector.tensor_tensor(tmp_a, tmp_b, tmp_b, ALU.mult)
                    nc.vector.tensor_tensor(tmp_a, tmp_a, vsrc, ALU.mult)
                    nc.vector.tensor_scalar(
                        out=tmp_a, in0=tmp_a, scalar1=-0.5, scalar2=1.5,
                        op0=ALU.mult, op1=ALU.add)
                    nc.vector.tensor_tensor(
                        dst if it == 1 else tmp_b, tmp_b, tmp_a, ALU.mult)

            mu1 = []
            rstd1 = []
            nmr1 = []
            for b in range(BL):
                mu1.append(stat.tile([P, NC_TOK], F32, name=f"mu{b}"))
                rstd1.append(stat.tile([P, NC_TOK], F32, name=f"rs{b}"))
                nmr1.append(stat.tile([P, NC_TOK], F32, name=f"nm{b}"))

            with (
                tc.tile_pool(name="pstat", bufs=1, space="PSUM") as pstat,
                tc.tile_pool(name="ps1", bufs=2, space="PSUM") as ps1,
                tc.tile_pool(name="pst", bufs=1, space="PSUM") as pst,
                tc.tile_pool(name="ps2", bufs=2, space="PSUM") as ps2,
                tc.tile_pool(name="vcp", bufs=1, space="PSUM") as vcp,
            ):
                vc = vcp.tile([P, BL * NC_TOK], F32, name="vc")
                h1c = [[None] * KTM for _ in range(BL)]
                sq_tiles = {}

                def emit_stats(b):
                    """LN1 per-token mean/rstd without DVE reductions."""
                    sqT = y2r[b]
                    nc.scalar.activation(sqT, xT_sb[b], AF.Square)
                    ps = pstat.tile([P, P], F32, tag="pstat")
                    for c in range(NC_TOK):
                        nc.tensor.matmul(
                            ps[:, c:c + 1],
                            xT_sb[b][:, c * P:(c + 1) * P], onesc_sb,
                            start=True, stop=True)
                    for c in range(NC_TOK):
                        nc.tensor.matmul(
                            ps[:, NC_TOK + c:NC_TOK + c + 1],
                            sqT[:, c * P:(c + 1) * P], onesc_sb,
                            start=True, stop=True)
                    vs = stat.tile([P, NC_TOK], F32, name=f"vs1_{b}")
                    ta = stat.tile([P, NC_TOK], F32, name=f"ta1_{b}")
                    tb = stat.tile([P, NC_TOK], F32, name=f"tb1_{b}")
                    # mu = sums/H ; var+eps = sumsq/H + EPS - mu^2
                    nc.vector.tensor_scalar_mul(mu1[b], ps[:, 0:NC_TOK],
                                                1.0 / H)
                    nc.vector.tensor_scalar(
                        out=ta, in0=ps[:, NC_TOK:], scalar1=1.0 / H,
                        scalar2=EPS, op0=ALU.mult, op1=ALU.add)
                    nc.vector.tensor_tensor(tb, mu1[b], mu1[b], ALU.mult)
                    nc.vector.tensor_tensor(vs, ta, tb, ALU.subtract)
                    rsqrt_chain(rstd1[b], vs, ta, tb)
                    # -mu*rstd, the per-chunk bias for the ACT xn path
                    nc.vector.tensor_tensor(ta, mu1[b], rstd1[b], ALU.mult)
                    nc.vector.tensor_scalar_mul(nmr1[b], ta, -1.0)

                def emit_mm1(b):
                    psum1 = ps1.tile([P, TM], F32, tag="ps1")
                    for c in range(NC_TOK):
                        xn = small.tile([P, P], BF16, tag="xn")
                        if c % 2 == 0:
                            nc.vector.tensor_scalar(
                                out=xn,
                                in0=x_sb[b][:, c, :],
                                scalar1=mu1[b][:, c:c + 1],
                                scalar2=rstd1[b][:, c:c + 1],
                                op0=ALU.subtract,
                                op1=ALU.mult,
                            )
                        else:
                            nc.scalar.activation(
                                xn, x_sb[b][:, c, :], AF.Identity,
                                bias=nmr1[b][:, c:c + 1],
                                scale=rstd1[b][:, c:c + 1])
                        nc.tensor.matmul(
                            psum1, xn, w1c_sb[:, c, :],
                            start=(c == 0), stop=(c == NC_TOK - 1))
                    return psum1

                def emit_ph2(b, psum1):
                    h1 = small.tile([P, TM], BF16, tag="h1")
                    if nontrivial_bias1:
                        h1f = small.tile([P, TM], F32, tag="h1f")
                        nc.vector.tensor_scalar_mul(h1f, psum1, g1_t)
                        nc.vector.tensor_add(h1f, h1f, bias1_sb)
                        nc.scalar.activation(h1, h1f, AF.Gelu)
                    else:
                        nc.scalar.activation(h1, psum1, AF.Gelu, scale=g1_t)
                    for k in range(KTM):
                        hk = h1[:, k * P:(k + 1) * P]
                        ps_t = pst.tile([P, P], BF16, tag="pst")
                        nc.tensor.transpose(ps_t, hk, ident)
                        hm = pst.tile([P, 1], F32, tag="hm")
                        nc.tensor.matmul(hm, hk, onesc_sb,
                                         start=True, stop=True)
                        nhm = small.tile([P, 1], F32, tag="nhm")
                        nc.scalar.activation(nhm, hm, AF.Copy,
                                             scale=float(-1.0 / H))
                        hc = small.tile([P, P], BF16, tag="h1c",
                                        name=f"hc{b}_{k}")
                        nc.scalar.activation(hc, ps_t, AF.Identity,
                                             bias=nhm)
                        h1c[b][k] = hc

                def emit_3a_mm(b, j, y2r_engine, sq_engine):
                    """token matmul 2 for (b, j) + psum drain + square."""
                    p2 = ps2.tile([P, JW], F32, tag="ps2")
                    for k in range(KTM):
                        nc.tensor.matmul(
                            p2, h1c[b][k], w2_sb[:, k, j, :],
                            start=(k == 0), stop=(k == KTM - 1))
                    yj = y2r[b][:, j * JW:(j + 1) * JW]
                    if y2r_engine == "act":
                        nc.scalar.activation(yj, p2, AF.Copy)
                    else:
                        nc.vector.tensor_copy(yj, p2)
                    sq = sqp.tile([P, JW], BF16, tag="sq")
                    if sq_engine == "gpsimd":
                        nc.gpsimd.tensor_tensor(sq, yj, yj, ALU.mult)
                    else:
                        nc.vector.tensor_tensor(sq, yj, yj, ALU.mult)
                    sq_tiles[(b, j)] = sq

                def emit_3a_vc(b, j):
                    """compact per-token variance columns for (b, j)."""
                    sq = sq_tiles.pop((b, j))
                    for c in range(TPJ):
                        col = b * NC_TOK + j * TPJ + c
                        nc.tensor.matmul(
                            vc[:, col:col + 1],
                            sq[:, c * P:(c + 1) * P], onesc_sb,
                            start=True, stop=True)

                def emit_chain2(b):
                    vs = stat.tile([P, NC_TOK], F32, name=f"vs2_{b}")
                    ta = stat.tile([P, NC_TOK], F32, name=f"ta2_{b}")
                    tb = stat.tile([P, NC_TOK], F32, name=f"tb2_{b}")
                    # padded to [P, P]: the XBAR transpose DMA needs a
                    # multiple-of-128 free dim
                    rc = stat.tile([P, P], BF16, name=f"rc2_{b}")
                    nc.gpsimd.memset(rc[:, NC_TOK:], 0.0)
                    nc.vector.tensor_scalar(
                        out=vs, in0=vc[:, b * NC_TOK:(b + 1) * NC_TOK],
                        scalar1=1.0 / H, scalar2=EPS,
                        op0=ALU.mult, op1=ALU.add)
                    rsqrt_chain(rc[:, 0:NC_TOK], vs, ta, tb)
                    # rc[t, c] --plain--> rs1 --transposed--> rr[c, t]
                    #   --plain--> rs2 --stride-0 broadcast--> bcast[p, c*t]
                    nc.sync.dma_start(rs1_d[b], rc)
                    rr = small.tile([P, P], BF16, tag="rr")
                    nc.sync.dma_start(rr, rs1_d[b], transpose=True)
                    nc.sync.dma_start(rs2_d[b], rr[0:NC_TOK, :])
                    nc.sync.dma_start(
                        bcast[b],
                        rs2_d[b].rearrange(
                            "c t -> (c t)").partition_broadcast(P))

                fr_tiles = {}

                def emit_3b_front(b, j, psr):
                    yn = ynp.tile([P, JW], BF16, tag="yn")
                    nc.vector.tensor_tensor(
                        yn, y2r[b][:, j * JW:(j + 1) * JW],
                        bcast[b][:, j * JW:(j + 1) * JW], ALU.mult)
                    raw = psr.tile([P, NCI * JW], F32, tag="psr")
                    for ci in range(NCI):
                        nc.tensor.matmul(
                            raw[:, ci * JW:(ci + 1) * JW],
                            cw1_sb[:, ci * P:(ci + 1) * P],
                            yn, start=True, stop=True)
                    g2 = g2p.tile([P, NCI * JW], BF16, tag="g2")
                    if nontrivial_bias1:
                        for ci in range(NCI):
                            nc.scalar.activation(
                                g2[:, ci * JW:(ci + 1) * JW],
                                raw[:, ci * JW:(ci + 1) * JW],
                                AF.Gelu, bias=cb1_sb[:, ci:ci + 1])
                    else:
                        nc.scalar.activation(g2, raw, AF.Gelu)
                    fr_tiles[(b, j)] = (raw, g2)

                def emit_3b_back(b, j):
                    raw, g2 = fr_tiles.pop((b, j))
                    # accumulate the output into bank 0 of `raw` (fully
                    # consumed by the gelu above) to stay in 8 banks
                    po = raw[:, 0:JW]
                    for ci in range(NCI):
                        nc.tensor.matmul(
                            po, cw2_sb[:, ci, :],
                            g2[:, ci * JW:(ci + 1) * JW],
                            start=(ci == 0), stop=(ci == NCI - 1))
                    osb = outp.tile([P, JW], F32, tag="osb")
                    if nontrivial_cb2:
                        nc.vector.tensor_scalar(
                            out=osb, in0=po, scalar1=cb2_sb,
                            scalar2=None, op0=ALU.add)
                    else:
                        nc.vector.tensor_copy(osb, po)
                    nc.sync.dma_start(out_v[b][:, j * JW:(j + 1) * JW], osb)

                # ---- emission schedule --------------------------------
                emit_stats(0)
                p1_0 = emit_mm1(0)
                emit_ph2(0, p1_0)
                emit_stats(1)
                for j in range(NJ):
                    emit_3a_mm(0, j, "act", "gpsimd")
                    if j > 0:
                        emit_3a_vc(0, j - 1)
                emit_3a_vc(0, NJ - 1)
                emit_chain2(0)
                p1_1 = emit_mm1(1)
                emit_ph2(1, p1_1)
                for j in range(NJ):
                    emit_3a_mm(1, j, "act", "dve")
                    if j > 0:
                        emit_3a_vc(1, j - 1)
                emit_3a_vc(1, NJ - 1)
                emit_chain2(1)

            # channel MLP for both batches: double-buffered 4-bank psum
            # tiles, with each iteration's output matmuls emitted AFTER
            # the next iteration's input matmuls so the in-order PE
            # queue never waits on a gelu it can overlap
            with (
                tc.tile_pool(name="psrA", bufs=1, space="PSUM") as psrA,
                tc.tile_pool(name="psrB", bufs=1, space="PSUM") as psrB,
            ):
                prev = None
                for b in range(BL):
                    for j in range(NJ):
                        emit_3b_front(b, j, psrA if (b * NJ + j) % 2
                                      else psrB)
                        if prev is not None:
                            emit_3b_back(*prev)
                        prev = (b, j)
                emit_3b_back(*prev)

    nc.compile()
    return nc


def _host_prep(inputs):
    import ml_dtypes

    x = np.asarray(inputs["x"], dtype=np.float32)
    ln1_g = np.asarray(inputs["ln1_g"], np.float32)
    ln1_b = np.asarray(inputs["ln1_b"], np.float32)
    ln2_g = np.asarray(inputs["ln2_g"], np.float32)
    ln2_b = np.asarray(inputs["ln2_b"], np.float32)
    tok_w1 = np.asarray(inputs["tok_w1"], np.float32)
    tok_b1 = np.asarray(inputs["tok_b1"], np.float32)
    tok_w2 = np.asarray(inputs["tok_w2"], np.float32)
    ch_w1 = np.asarray(inputs["ch_w1"], np.float32)
    ch_b1 = np.asarray(inputs["ch_b1"], np.float32)
    ch_w2 = np.asarray(inputs["ch_w2"], np.float32)
    ch_b2 = np.asarray(inputs["ch_b2"], np.float32)

    BF = ml_dtypes.bfloat16
    w1c = np.cumsum(tok_w1, axis=0, dtype=np.float64).astype(np.float32)
    colsum1 = w1c.sum(axis=0, dtype=np.float64).astype(np.float32)
    bias1 = ln1_b[:, None] * colsum1[None, :] + tok_b1[None, :]
    cw1 = (ln2_g[:, None] * ch_w1).astype(np.float32)
    cb1 = (ch_b1 + ch_w1.T @ ln2_b).astype(np.float32)

    nontrivial_bias1 = bool(np.any(bias1 != 0.0) or np.any(cb1 != 0.0))
    nontrivial_cb2 = bool(np.any(ch_b2 != 0.0))

    shared = {
        "w1c": w1c.astype(BF),
        "w2": np.ascontiguousarray(tok_w2).astype(BF),
        "g1": ln1_g.reshape(P, 1).copy(),
        "bias1": np.ascontiguousarray(bias1, np.float32),
        "cw1": cw1.astype(BF),
        "cb1": np.ascontiguousarray(cb1.reshape(NCI, P).T.copy()),
        "cw2": np.ascontiguousarray(ch_w2).astype(BF),
        "cb2": ch_b2.reshape(P, 1).astype(np.float32).copy(),
        "onesc": np.ones((P, 1), BF),
    }
    return x.astype(BF), shared, nontrivial_bias1, nontrivial_cb2


def kernel(**inputs) -> np.ndarray:
    from concourse.bass_utils import run_bass_kernel_spmd

    x, shared, nb1, nb2 = _host_prep(inputs)

    key = (nb1, nb2)
    if key not in _cached:
        _cached[key] = _build(nb1, nb2)
    nc = _cached[key]

    in_maps = []
    for c in range(NCORES):
        m = dict(shared)
        m["x"] = np.ascontiguousarray(x[c * BL:(c + 1) * BL])
        in_maps.append(m)

    res = run_bass_kernel_spmd(nc, in_maps, core_ids=list(range(NCORES)))
    out = np.concatenate(
        [r["out"].transpose(0, 2, 1) for r in res.results], axis=0)
    return np.ascontiguousarray(out, dtype=np.float32)


if __name__ == "__main__":
    rng = np.random.default_rng(0)
    ins = {
        "x": rng.standard_normal((B, N, H)).astype(np.float32),
        "ln1_g": np.ones(H, np.float32),
        "ln1_b": np.zeros(H, np.float32),
        "ln2_g": np.ones(H, np.float32),
        "ln2_b": np.zeros(H, np.float32),
        "tok_w1": (rng.standard_normal((N, TM)) * 0.02).astype(np.float32),
        "tok_b1": np.zeros(TM, np.float32),
        "tok_w2": (rng.standard_normal((TM, N)) * 0.02).astype(np.float32),
        "tok_b2": np.zeros(N, np.float32),
        "ch_w1": (rng.standard_normal((H, CM)) * 0.02).astype(np.float32),
        "ch_b1": np.zeros(CM, np.float32),
        "ch_w2": (rng.standard_normal((CM, H)) * 0.02).astype(np.float32),
        "ch_b2": np.zeros(H, np.float32),
    }
    out = kernel(**ins)
    print("out", out.shape, out.dtype)
